# revision 23
# baseline (speedup 1.0000x reference)
import os
import sys

for _p in ("/opt/trn_rl_repo", "/root/.axon_site/_ro/trn_rl_repo"):
    if os.path.isdir(_p) and _p not in sys.path:
        sys.path.append(_p)

import numpy as np

P = 128
HW = 256 * 256
NCLS = 21
CPAD = 32
CH = 128
GRP = 512
NG = HW // GRP
FB = 2048
NB = HW // FB
EPS = 1e-8
N_CORES = 8

_CACHE = {}


def _build_module(variant="full"):
    import concourse.bacc as bacc
    import concourse.mybir as mybir
    import concourse.tile as tile
    from concourse.masks import make_identity

    dma_only = variant == "dma"

    f32 = mybir.dt.float32
    bf16 = mybir.dt.bfloat16
    i32 = mybir.dt.int32
    EQ = mybir.AluOpType.is_equal
    MULT = mybir.AluOpType.mult

    nc = bacc.Bacc("TRN2", target_bir_lowering=False, debug=False)
    img = nc.dram_tensor("img", [P, HW], f32, kind="ExternalInput")
    gt = nc.dram_tensor("gt", [HW], i32, kind="ExternalInput")
    out = nc.dram_tensor("out", [P, HW], bf16, kind="ExternalOutput")

    with tile.TileContext(nc) as tc:
        with (
            tc.tile_pool(name="constp", bufs=1) as constp,
            tc.tile_pool(name="imgp", bufs=6) as imgp,
            tc.tile_pool(name="bfp", bufs=4) as bfp,
            tc.tile_pool(name="rhsp", bufs=10) as rhsp,
            tc.tile_pool(name="ohp", bufs=12) as ohp,
            tc.tile_pool(name="stashp", bufs=1) as stashp,
            tc.tile_pool(name="outp", bufs=4) as outp,
            tc.tile_pool(name="psA", bufs=4, space="PSUM") as psA,
            tc.tile_pool(name="psT", bufs=2, space="PSUM") as psT,
            tc.tile_pool(name="psB", bufs=1, space="PSUM") as psB,
        ):
            if dma_only:
                zb = constp.tile([P, FB], bf16, tag="zb")
                nc.vector.memset(zb[:], 0.0)
                for t in range(NB):
                    ib = imgp.tile([P, FB], f32, tag="img")
                    for jj in range(2):
                        nc.sync.dma_start(
                            out=ib[:, jj * 1024 : (jj + 1) * 1024],
                            in_=img.ap()[
                                :, t * FB + jj * 1024 : t * FB + (jj + 1) * 1024
                            ],
                        )
                for t in range(NB):
                    nc.sync.dma_start(out=out.ap()[:, t * FB : (t + 1) * FB], in_=zb[:])
                nc.compile()
                return nc

            ident32 = constp.tile([P, P], f32, tag="id32")
            make_identity(nc, ident32[:])
            ident16 = constp.tile([P, P], bf16, tag="id16")
            nc.vector.tensor_copy(out=ident16[:], in_=ident32[:])
            iota4 = constp.tile([P, P], bf16, tag="iota4")
            for c in range(CPAD):
                nc.vector.memset(iota4[:, c : c + 1], float(c))
            for r in range(1, 4):
                nc.vector.tensor_copy(
                    out=iota4[:, r * CPAD : (r + 1) * CPAD], in_=iota4[:, 0:CPAD]
                )
            ones1 = constp.tile([P, 1], bf16, tag="ones1")
            nc.vector.memset(ones1[:], 1.0)

            s01 = constp.tile([P, CPAD], f32, tag="s01")
            nc.vector.tensor_add(s01[:], ident32[:, 0:32], ident32[:, 32:64])
            s23 = constp.tile([P, CPAD], f32, tag="s23")
            nc.vector.tensor_add(s23[:], ident32[:, 64:96], ident32[:, 96:128])
            selF = constp.tile([P, CPAD], f32, tag="selF")
            nc.vector.tensor_add(selF[:], s01[:], s23[:])
            sel2ps = psA.tile([CPAD, P], f32, tag="a")
            nc.tensor.transpose(out=sel2ps[:], in_=selF[:], identity=ident32[:])
            sel2 = constp.tile([CPAD, P], bf16, tag="sel2")
            nc.vector.tensor_copy(out=sel2[:], in_=sel2ps[:])

            gtn_i = constp.tile([32, HW // 32], i32, tag="gtn_i")
            nc.sync.dma_start(out=gtn_i[:], in_=gt.ap().rearrange("(p f) -> p f", p=32))
            gtnb = constp.tile([32, HW // 32], bf16, tag="gtnb")
            for s in range(4):
                eng = (nc.vector, nc.scalar, nc.gpsimd, nc.vector)[s]
                cv = eng.copy if eng is nc.scalar else eng.tensor_copy
                cv(
                    out=gtnb[:, s * 512 : (s + 1) * 512],
                    in_=gtn_i[:, s * 512 : (s + 1) * 512],
                )
            gtT = constp.tile([P, HW // CH + 32], bf16, tag="gtT")
            for b in range(16):
                gps = psT.tile([P, 32], bf16, tag="t")
                nc.tensor.transpose(
                    out=gps[:],
                    in_=gtnb[:, b * P : (b + 1) * P],
                    identity=ident16[0:32, 0:32],
                )
                if b % 2 == 0:
                    nc.vector.tensor_copy(out=gtT[:, b * 32 : (b + 1) * 32], in_=gps[:])
                else:
                    nc.scalar.copy(out=gtT[:, b * 32 : (b + 1) * 32], in_=gps[:])

            ohsT = stashp.tile([P, CH * NG], bf16, tag="ohsT")

            sumsT = psB.tile([CPAD, P], f32, tag="sums")
            sums = sumsT[0:CPAD, 0:P]
            cnt = psB.tile([P, 130], f32, tag="cnt")
            cntG = cnt[:, 0:1]

            ohs = {}
            rhss = {}
            ibbs = {}
            for t in range(NB + 2):
                if t < NB:
                    ib = imgp.tile([P, FB], f32, tag="img")
                    ibb = bfp.tile([P, FB], bf16, tag="ibb")
                    for jj in range(4):
                        nc.sync.dma_start(
                            out=ib[:, jj * GRP : (jj + 1) * GRP],
                            in_=img.ap()[
                                :, t * FB + jj * GRP : t * FB + (jj + 1) * GRP
                            ],
                        )
                    for j in range(4):
                        g = t * 4 + j
                        base = 32 * ((4 * g) % 16) + g // 4
                        oh = ohp.tile([P, P], bf16, tag="oh")
                        ohs[g] = oh
                        lab3 = (
                            gtT[:, base : base + 128]
                            .rearrange("p (a b) -> p a b", a=4, b=32)[:, :, 0:1]
                            .broadcast_to([P, 4, CPAD])
                        )
                        nc.gpsimd.tensor_tensor(
                            out=oh[:].rearrange("p (a b) -> p a b", a=4),
                            in0=iota4[:].rearrange("p (a b) -> p a b", a=4),
                            in1=lab3,
                            op=EQ,
                        )
                if 1 <= t <= NB:
                    tm = t - 1
                    ohT4 = psT.tile([P, GRP], bf16, tag="t")
                    tp2 = None
                    for j in range(4):
                        g = tm * 4 + j
                        if j % 2 == 0:
                            tp2 = psA.tile([P, 2 * GRP], bf16, tag="a")
                        half = (j % 2) * GRP
                        for q in range(4):
                            nc.tensor.transpose(
                                out=tp2[:, half + q * CH : half + (q + 1) * CH],
                                in_=ibbs[tm][:, (j * 4 + q) * CH : (j * 4 + q + 1) * CH],
                                identity=ident16[:],
                            )
                        nc.tensor.transpose(
                            out=ohT4[:, j * P : (j + 1) * P],
                            in_=ohs[g][:],
                            identity=ident16[:],
                        )
                        rhs = rhsp.tile([P, GRP], bf16, tag="rhs")
                        rhss[g] = rhs
                        nc.vector.tensor_copy(
                            out=rhs[:], in_=tp2[:, half : half + GRP]
                        )
                    nc.scalar.copy(
                        out=ohsT[:, tm * GRP : (tm + 1) * GRP], in_=ohT4[:]
                    )
                if t >= 2:
                    tm = t - 2
                    for j in range(4):
                        g = tm * 4 + j
                        oh = ohs.pop(g)
                        rhs = rhss.pop(g)
                        nc.tensor.matmul(
                            out=cntG,
                            lhsT=oh[:],
                            rhs=ones1[:],
                            start=(g == 0),
                            stop=(g == NG - 1),
                        )
                        for q in range(4):
                            gc = g * 4 + q
                            nc.tensor.matmul(
                                out=sums,
                                lhsT=oh[:, q * CPAD : (q + 1) * CPAD],
                                rhs=rhs[:, q * CH : (q + 1) * CH],
                                start=(gc == 0),
                                stop=(gc == HW // CH - 1),
                            )
                if t < NB:
                    nc.gpsimd.tensor_copy(out=ibb[:, 0:GRP], in_=ib[:, 0:GRP])
                    nc.vector.tensor_copy(
                        out=ibb[:, GRP : 2 * GRP], in_=ib[:, GRP : 2 * GRP]
                    )
                    nc.scalar.copy(
                        out=ibb[:, 2 * GRP : 3 * GRP], in_=ib[:, 2 * GRP : 3 * GRP]
                    )
                    nc.scalar.copy(
                        out=ibb[:, 3 * GRP : 4 * GRP], in_=ib[:, 3 * GRP : 4 * GRP]
                    )
                    ibbs[t] = ibb

            cntSB = constp.tile([P, 1], f32, tag="cntSB")
            nc.vector.tensor_copy(out=cntSB[:], in_=cntG)
            cnt32 = cnt[0:CPAD, 1:2]
            nc.tensor.matmul(
                out=cnt32, lhsT=selF[:], rhs=cntSB[:], start=True, stop=True
            )
            cntE = constp.tile([CPAD, 1], f32, tag="cntE")
            nc.vector.tensor_scalar_add(cntE[:], cnt32, EPS)
            rcp = constp.tile([CPAD, 1], f32, tag="rcp")
            nc.vector.reciprocal(out=rcp[:], in_=cntE[:])
            means = constp.tile([CPAD, P], bf16, tag="means")
            nc.vector.tensor_scalar(means[:], sums, rcp[:, 0:1], None, MULT)
            mqps = cnt[:, 2 : P + 2]
            nc.tensor.matmul(
                out=mqps, lhsT=sel2[:], rhs=means[:], start=True, stop=True
            )
            meansQ = constp.tile([P, P], bf16, tag="meansQ")
            nc.vector.tensor_copy(out=meansQ[:], in_=mqps)
            mq4 = constp.tile([P, 4 * P], bf16, tag="mq4")
            nc.vector.memset(mq4[:], 0.0)
            for q in range(4):
                nc.vector.tensor_copy(
                    out=mq4[q * CPAD : (q + 1) * CPAD, q * P : (q + 1) * P],
                    in_=meansQ[q * CPAD : (q + 1) * CPAD, :],
                )

            for t in range(NB):
                ob = outp.tile([P, FB], bf16, tag="ob")
                for j in range(4):
                    g = t * 4 + j
                    ops = psA.tile([P, GRP], f32, tag="a")
                    for q in range(4):
                        nc.tensor.matmul(
                            out=ops[:, q * CH : (q + 1) * CH],
                            lhsT=mq4[:, q * P : (q + 1) * P],
                            rhs=ohsT[:, g * CH : (g + 1) * CH],
                            start=True,
                            stop=True,
                        )
                    if j % 2 == 0:
                        nc.vector.tensor_copy(
                            out=ob[:, j * GRP : (j + 1) * GRP], in_=ops[:]
                        )
                    else:
                        nc.scalar.copy(out=ob[:, j * GRP : (j + 1) * GRP], in_=ops[:])
                nc.sync.dma_start(out=out.ap()[:, t * FB : (t + 1) * FB], in_=ob[:])

    nc.compile()
    return nc


def get_module():
    if "nc" not in _CACHE:
        _CACHE["nc"] = _build_module()
    return _CACHE["nc"]


def kernel(img, gt):
    from concourse.bass_utils import run_bass_kernel_spmd

    img = np.asarray(img)
    gt = np.asarray(gt)
    B, C, H, W = img.shape
    assert (B, C, H * W) == (N_CORES, P, HW), (img.shape,)
    img2 = np.ascontiguousarray(img.reshape(B, C, H * W))
    gt2 = np.ascontiguousarray(gt.reshape(B, H * W))

    nc = get_module()
    in_maps = [{"img": img2[i], "gt": gt2[i]} for i in range(B)]
    res = run_bass_kernel_spmd(nc, in_maps, core_ids=list(range(B)))
    out = np.stack(
        [np.asarray(res.results[i]["out"]).astype(np.float32) for i in range(B)],
        axis=0,
    )
    return out.reshape(B, C, H, W)


if __name__ == "__main__":
    rng = np.random.default_rng(0)
    img = rng.standard_normal((8, 128, 256, 256), dtype=np.float32)
    gt = rng.integers(0, NCLS, size=(8, 1, 256, 256), dtype=np.int32)
    out = kernel(img=img, gt=gt)
    print("out", out.shape, out.dtype)
